# revision 1
# baseline (speedup 1.0000x reference)
"""AttentionGate3D Trainium2 kernel.

Computes out = x * sigmoid(Wpsi @ relu(Wg@g + bg + Wx@x + bx) + bpsi) for
g, x of shape [2, 512, 32, 64, 64] (NCDHW); the convs are 1x1x1, i.e.
per-voxel channel matmuls.

Sharding: depth D=32 is split across the 8 NeuronCores (4 d-slices per
batch per core); 1x1x1 convs are per-voxel so no halo exchange is needed
and the (tiny) weights are replicated to every core.

Per-core layout is [B=2, C=512, V=16384] (channels x flat voxels), fp16.
fp16 keeps 10 mantissa bits (measured end-to-end scale-relative error
~9e-4 vs the fp32 reference) while halving HBM traffic vs fp32 and
running the PE at 1 cycle/row — the kernel is jointly limited by the
tensor engine (~253us busy) and HBM (~96MB/core).

Structure per 2048-voxel block (one 2MB DMA per tensor; loads on the
sync HWDGE ring, stores on the scalar HWDGE ring so stores never stall
loads; weights prefetched via SWDGE):
  - per 512-voxel chunk: 16 accumulating fp16 matmuls (Wg@g and Wx@x
    share one PSUM accumulation per 128-channel output block), then a
    fused bias+ReLU on the scalar engine (PSUM -> fp16 SBUF).
  - psi matmuls use Wpsi replicated across all 128 stationary columns,
    so psi lands in PSUM already broadcast across partitions; they are
    emitted one chunk late so the PE never waits on the relu.
  - per 1024 voxels: fused bias+Sigmoid, then one DVE multiply
    out = x * attn over [128, 4*1024] with attn broadcast along the
    channel-chunk dim (2 elem/cycle fp16 mode).
"""

import sys

sys.path.insert(0, "/opt/trn_rl_repo")

import numpy as np

import concourse.bass as bass
import concourse.tile as tile
from concourse import bacc, mybir
from concourse.bass_utils import run_bass_kernel_spmd

N_CORES = 8
B, C, D, H, W = 2, 512, 32, 64, 64
F_INT = 256
D_PER_CORE = D // N_CORES           # 4
V = D_PER_CORE * H * W              # 16384 voxels per batch per core
VB = 2048                           # voxels per DMA block
CHUNK = 512                         # voxels per PSUM-bank matmul
KC = C // 128                       # 4 contraction chunks
MC = F_INT // 128                   # 2 output-channel chunks
PSI_KC = F_INT // 128               # 2 psi contraction chunks

F32 = mybir.dt.float32
F16 = mybir.dt.float16

_cache: dict = {}


def _build(vb: int = VB, v: int = V, x_swdge: bool = False, deep_io: bool = True, store_swdge: bool = False):
    nc = bacc.Bacc(
        "TRN2", target_bir_lowering=False, debug=False, num_devices=N_CORES
    )

    g_d = nc.declare_dram_parameter("g", [B, C, v], F16, isOutput=False)
    x_d = nc.declare_dram_parameter("x", [B, C, v], F16, isOutput=False)
    wg_d = nc.declare_dram_parameter("wg", [128, KC, F_INT], F16, isOutput=False)
    wx_d = nc.declare_dram_parameter("wx", [128, KC, F_INT], F16, isOutput=False)
    wpsi_d = nc.declare_dram_parameter("wpsi", [128, PSI_KC, 128], F16, isOutput=False)
    bgx_d = nc.declare_dram_parameter("bgx", [128, MC + 1], F32, isOutput=False)
    out_d = nc.declare_dram_parameter("out", [B, C, v], F16, isOutput=True)

    # [p, kc, v] views of the [B, C, v] tensors (channel c = kc*128 + p)
    g_v = g_d.rearrange("b (kc p) v -> b p kc v", p=128)
    x_v = x_d.rearrange("b (kc p) v -> b p kc v", p=128)
    out_v = out_d.rearrange("b (kc p) v -> b p kc v", p=128)

    n_vb = v // vb
    n_chunks = vb // CHUNK

    with tile.TileContext(nc) as tc:
        with (
            tc.tile_pool(name="wpool", bufs=1) as wpool,
            tc.tile_pool(name="io", bufs=4 if deep_io else 3) as io,
            tc.tile_pool(name="op", bufs=2 if deep_io else 3) as op,
            tc.tile_pool(name="act", bufs=4) as actp,
            tc.tile_pool(name="psum", bufs=2, space="PSUM") as psum,
        ):
            wg_sb = wpool.tile([128, KC, F_INT], F16)
            wx_sb = wpool.tile([128, KC, F_INT], F16)
            wpsi_sb = wpool.tile([128, PSI_KC, 128], F16)
            bgx_sb = wpool.tile([128, MC + 1], F32)
            nc.gpsimd.dma_start(wg_sb[:], wg_d[:])
            nc.gpsimd.dma_start(wx_sb[:], wx_d[:])
            nc.gpsimd.dma_start(wpsi_sb[:], wpsi_d[:])
            nc.gpsimd.dma_start(bgx_sb[:], bgx_d[:])

            # psi matmuls run one chunk late (so relu is long done when the
            # PE reaches them); sigmoid + gating multiply run per 2 chunks.
            pending = []               # [(relu tiles, chunk idx)]
            epilog = []                # [(psi psum, x tile, out tile, j0, width)]
            psi_state = [None, 0]      # (pspsi tile, base chunk), chunks filled

            def flush_epilog():
                for (ps_psi, x_t, o_t, j0, width) in epilog:
                    attn = actp.tile([128, 2 * CHUNK], F16, tag="attn")
                    nc.scalar.activation(
                        attn[:, :width],
                        ps_psi[:, :width],
                        mybir.ActivationFunctionType.Sigmoid,
                        bias=bgx_sb[:, MC : MC + 1],
                    )
                    vs2 = bass.ds(j0 * CHUNK, width)
                    nc.vector.tensor_mul(
                        o_t[:, :, vs2],
                        x_t[:, :, vs2],
                        attn[:, :width].unsqueeze(1).to_broadcast([128, KC, width]),
                    )
                epilog.clear()

            def flush_pending():
                for (relu_t, js) in pending:
                    if psi_state[0] is None:
                        ps_psi_new = psum.tile(
                            [128, 2 * CHUNK], F32, tag="pspsi", name="ps_psi"
                        )
                        psi_state[0] = (ps_psi_new, js)
                        psi_state[1] = 0
                    ps_psi, j0 = psi_state[0]
                    off = psi_state[1] * CHUNK
                    for m in range(PSI_KC):
                        nc.tensor.matmul(
                            ps_psi[:, off : off + CHUNK],
                            wpsi_sb[:, m, :],
                            relu_t[m][:],
                            start=(m == 0),
                            stop=(m == PSI_KC - 1),
                        )
                    psi_state[1] += 1
                pending.clear()

            def close_psi(x_t, o_t):
                if psi_state[0] is not None:
                    ps_psi, j0 = psi_state[0]
                    epilog.append((ps_psi, x_t, o_t, j0, psi_state[1] * CHUNK))
                    psi_state[0] = None
                    psi_state[1] = 0

            for b in range(B):
                for i in range(n_vb):
                    vs = bass.ds(i * vb, vb)
                    g_t = io.tile([128, KC, vb], F16, tag="g")
                    nc.sync.dma_start(g_t[:], g_v[b, :, :, vs])
                    x_t = io.tile([128, KC, vb], F16, tag="x")
                    (nc.gpsimd if x_swdge else nc.sync).dma_start(
                        x_t[:], x_v[b, :, :, vs]
                    )
                    o_t = op.tile([128, KC, vb], F16, tag="o")

                    for j in range(n_chunks):
                        cs = bass.ts(j, CHUNK)
                        relu_t = []
                        for m in range(MC):
                            ps = psum.tile([128, CHUNK], F32, tag=f"ps{m}")
                            for kc in range(KC):
                                nc.tensor.matmul(
                                    ps[:],
                                    wg_sb[:, kc, bass.ts(m, 128)],
                                    g_t[:, kc, cs],
                                    start=(kc == 0),
                                    stop=False,
                                )
                            for kc in range(KC):
                                nc.tensor.matmul(
                                    ps[:],
                                    wx_sb[:, kc, bass.ts(m, 128)],
                                    x_t[:, kc, cs],
                                    start=False,
                                    stop=(kc == KC - 1),
                                )
                            rt = actp.tile([128, CHUNK], F16, tag=f"relu{m}")
                            nc.scalar.activation(
                                rt[:],
                                ps[:],
                                mybir.ActivationFunctionType.Relu,
                                bias=bgx_sb[:, m : m + 1],
                            )
                            relu_t.append(rt)
                        flush_pending()
                        if j % 2 == 0 and j > 0:
                            close_psi(x_t, o_t)
                            flush_epilog()
                        pending.append((relu_t, j))

                    flush_pending()
                    close_psi(x_t, o_t)
                    flush_epilog()
                    (nc.gpsimd if store_swdge else nc.scalar).dma_start(
                        out_v[b, :, :, vs], o_t[:]
                    )

    nc.compile()
    return nc


def _prep_weights(Wg, bg, Wx, bx, Wpsi, bpsi_val):
    # wg[p, kc, m] = Wg[m, kc*128 + p] (stationary lhsT chunks)
    wg = np.ascontiguousarray(
        Wg.T.reshape(KC, 128, F_INT).transpose(1, 0, 2)
    ).astype(np.float16)
    wx = np.ascontiguousarray(
        Wx.T.reshape(KC, 128, F_INT).transpose(1, 0, 2)
    ).astype(np.float16)
    # wpsi[p, m_chunk, :] = Wpsi[0, m_chunk*128 + p], replicated across all
    # 128 stationary columns so psi lands broadcast across partitions.
    wp = Wpsi[0].reshape(PSI_KC, 128).T
    wpsi = np.ascontiguousarray(np.repeat(wp[:, :, None], 128, axis=2)).astype(
        np.float16
    )
    bgx = np.empty((128, MC + 1), dtype=np.float32)
    bgx[:, :MC] = (bg + bx).reshape(MC, 128).T
    bgx[:, MC] = bpsi_val
    return wg, wx, wpsi, bgx


def kernel(g, x, Wg, bg, Wx, bx, Wpsi, bpsi, _trace=False):
    if "nc" not in _cache:
        _cache["nc"] = _build()
    nc = _cache["nc"]

    g = np.asarray(g, dtype=np.float32)
    x = np.asarray(x, dtype=np.float32)
    bpsi_val = float(np.asarray(bpsi).reshape(-1)[0])
    wg, wx, wpsi, bgx = _prep_weights(
        np.asarray(Wg, np.float32),
        np.asarray(bg, np.float32),
        np.asarray(Wx, np.float32),
        np.asarray(bx, np.float32),
        np.asarray(Wpsi, np.float32),
        bpsi_val,
    )
    in_maps = []
    for k in range(N_CORES):
        sl = slice(k * D_PER_CORE, (k + 1) * D_PER_CORE)
        in_maps.append(
            {
                "g": np.ascontiguousarray(g[:, :, sl])
                .reshape(B, C, V)
                .astype(np.float16),
                "x": np.ascontiguousarray(x[:, :, sl])
                .reshape(B, C, V)
                .astype(np.float16),
                "wg": wg,
                "wx": wx,
                "wpsi": wpsi,
                "bgx": bgx,
            }
        )
    try:
        res = run_bass_kernel_spmd(nc, in_maps, list(range(N_CORES)), trace=_trace)
    except Exception:
        # transient axon/PJRT hiccups have been observed; one retry
        res = run_bass_kernel_spmd(nc, in_maps, list(range(N_CORES)), trace=_trace)

    out = np.empty((B, C, D, H, W), dtype=np.float32)
    for k in range(N_CORES):
        sl = slice(k * D_PER_CORE, (k + 1) * D_PER_CORE)
        out[:, :, sl] = (
            res.results[k]["out"].astype(np.float32).reshape(B, C, D_PER_CORE, H, W)
        )
    if _trace:
        return out, res
    return out



# revision 4
# speedup vs baseline: 1.2758x; 1.2758x over previous
"""AttentionGate3D Trainium2 kernel.

Computes out = x * sigmoid(Wpsi @ relu(Wg@g + bg + Wx@x + bx) + bpsi) for
g, x of shape [2, 512, 32, 64, 64] (NCDHW); the convs are 1x1x1, i.e.
per-voxel channel matmuls.

Sharding: depth D=32 is split across the 8 NeuronCores (4 d-slices per
batch per core); 1x1x1 convs are per-voxel so no halo exchange is needed
and the (tiny) weights are replicated to every core.

Per-core layout is [B=2, C=512, V=16384] (channels x flat voxels).
The g path runs in fp8e4m3 (data + Wg) with DoubleRow matmuls: one
instruction contracts 2x128 channels at the fp8 rate, halving both the
g HBM traffic (16.8 MB vs 33.5) and the PE time of the g conv. x stays
fp16 end-to-end: the gating multiply out = x * attn needs ~0.05% x
fidelity, and the loaded fp16 copy also feeds the Wx conv at the fp16
PE rate. psi stays fp16 (its PE share is small and fp16 protects the
attention logits). HBM/core: 16.8(g) + 33.5(x) + 33.5(out) = 84 MB.

DMA queues: x loads on the sync HWDGE ring, g loads + weights on the
gpsimd SWDGE rings, out stores on the scalar ring — no ring carries
more than 33.5 MB.

Structure per 2048-voxel block:
  - per 512-voxel chunk and 128-channel output block: 2 fp8 DoubleRow
    matmuls (Wg@g) + 4 fp16 matmuls (Wx@x) share one PSUM accumulation,
    then a fused bias+ReLU on the scalar engine (PSUM -> fp16 SBUF).
  - psi matmuls use Wpsi replicated across all 128 stationary columns,
    so psi lands in PSUM already broadcast across partitions; they are
    emitted one chunk late so the PE never waits on the relu.
  - per 1024 voxels: fused bias+Sigmoid, then one DVE multiply
    out = x * attn over [128, 4*1024] with attn broadcast along the
    channel-chunk dim (2 elem/cycle fp16 mode).
"""

import sys

sys.path.insert(0, "/opt/trn_rl_repo")

import ml_dtypes
import numpy as np

import concourse.bass as bass
import concourse.tile as tile
from concourse import bacc, mybir
from concourse.bass_utils import run_bass_kernel_spmd

N_CORES = 8
B, C, D, H, W = 2, 512, 32, 64, 64
F_INT = 256
D_PER_CORE = D // N_CORES           # 4
V = D_PER_CORE * H * W              # 16384 voxels per batch per core
VB = 2048                           # voxels per DMA block
CHUNK = 512                         # voxels per PSUM-bank matmul
KC = C // 128                       # 4 contraction chunks
KP = KC // 2                        # 2 DoubleRow pair-chunks for the g conv
MC = F_INT // 128                   # 2 output-channel chunks
PSI_KC = F_INT // 128               # 2 psi contraction chunks

F32 = mybir.dt.float32
F16 = mybir.dt.float16
F8 = mybir.dt.float8e4
NP_F8 = ml_dtypes.float8_e4m3

_cache: dict = {}


def _build(vb: int = VB, v: int = V):
    nc = bacc.Bacc(
        "TRN2", target_bir_lowering=False, debug=False, num_devices=N_CORES
    )

    g_d = nc.declare_dram_parameter("g", [B, C, v], F8, isOutput=False)
    x_d = nc.declare_dram_parameter("x", [B, C, v], F16, isOutput=False)
    wg_d = nc.declare_dram_parameter("wg", [128, KC, F_INT], F8, isOutput=False)
    wx_d = nc.declare_dram_parameter("wx", [128, KC, F_INT], F16, isOutput=False)
    wpsi_d = nc.declare_dram_parameter("wpsi", [128, PSI_KC, 128], F16, isOutput=False)
    bgx_d = nc.declare_dram_parameter("bgx", [128, MC + 1], F32, isOutput=False)
    out_d = nc.declare_dram_parameter("out", [B, C, v], F16, isOutput=True)

    # [p, kc, v] views of the [B, C, v] tensors (channel c = kc*128 + p)
    g_v = g_d.rearrange("b (kc p) v -> b p kc v", p=128)
    x_v = x_d.rearrange("b (kc p) v -> b p kc v", p=128)
    out_v = out_d.rearrange("b (kc p) v -> b p kc v", p=128)

    n_vb = v // vb
    n_chunks = vb // CHUNK

    with tile.TileContext(nc) as tc:
        with (
            tc.tile_pool(name="wpool", bufs=1) as wpool,
            tc.tile_pool(name="io", bufs=4) as io,
            tc.tile_pool(name="op", bufs=2) as op,
            tc.tile_pool(name="act", bufs=4) as actp,
            tc.tile_pool(name="psum", bufs=2, space="PSUM") as psum,
        ):
            wg_sb = wpool.tile([128, KC, F_INT], F8)
            wx_sb = wpool.tile([128, KC, F_INT], F16)
            wpsi_sb = wpool.tile([128, PSI_KC, 128], F16)
            bgx_sb = wpool.tile([128, MC + 1], F32)
            nc.gpsimd.dma_start(wg_sb[:], wg_d[:])
            nc.gpsimd.dma_start(wx_sb[:], wx_d[:])
            nc.gpsimd.dma_start(wpsi_sb[:], wpsi_d[:])
            nc.gpsimd.dma_start(bgx_sb[:], bgx_d[:])

            # psi matmuls run one chunk late (so relu is long done when the
            # PE reaches them); sigmoid + gating multiply run per 2 chunks.
            pending = []               # [(relu tiles, chunk idx)]
            epilog = []                # [(psi psum, x tile, out tile, j0, width)]
            psi_state = [None, 0]      # (pspsi tile, base chunk), chunks filled

            def flush_epilog():
                for (ps_psi, x_t, o_t, j0, width) in epilog:
                    attn = actp.tile([128, 2 * CHUNK], F16, tag="attn")
                    nc.scalar.activation(
                        attn[:, :width],
                        ps_psi[:, :width],
                        mybir.ActivationFunctionType.Sigmoid,
                        bias=bgx_sb[:, MC : MC + 1],
                    )
                    vs2 = bass.ds(j0 * CHUNK, width)
                    nc.vector.tensor_mul(
                        o_t[:, :, vs2],
                        x_t[:, :, vs2],
                        attn[:, :width].unsqueeze(1).to_broadcast([128, KC, width]),
                    )
                epilog.clear()

            def flush_pending():
                for (relu_t, js) in pending:
                    if psi_state[0] is None:
                        ps_psi_new = psum.tile(
                            [128, 2 * CHUNK], F32, tag="pspsi", name="ps_psi"
                        )
                        psi_state[0] = (ps_psi_new, js)
                        psi_state[1] = 0
                    ps_psi, j0 = psi_state[0]
                    off = psi_state[1] * CHUNK
                    for m in range(PSI_KC):
                        nc.tensor.matmul(
                            ps_psi[:, off : off + CHUNK],
                            wpsi_sb[:, m, :],
                            relu_t[m][:],
                            start=(m == 0),
                            stop=(m == PSI_KC - 1),
                        )
                    psi_state[1] += 1
                pending.clear()

            def close_psi(x_t, o_t):
                if psi_state[0] is not None:
                    ps_psi, j0 = psi_state[0]
                    epilog.append((ps_psi, x_t, o_t, j0, psi_state[1] * CHUNK))
                    psi_state[0] = None
                    psi_state[1] = 0

            for b in range(B):
                for i in range(n_vb):
                    vs = bass.ds(i * vb, vb)
                    g_t = io.tile([128, KC, vb], F8, tag="g")
                    nc.gpsimd.dma_start(g_t[:], g_v[b, :, :, vs])
                    x_t = io.tile([128, KC, vb], F16, tag="x")
                    nc.sync.dma_start(x_t[:], x_v[b, :, :, vs])
                    o_t = op.tile([128, KC, vb], F16, tag="o")

                    for j in range(n_chunks):
                        cs = bass.ts(j, CHUNK)
                        relu_t = []
                        for m in range(MC):
                            ps = psum.tile([128, CHUNK], F32, tag=f"ps{m}")
                            ms = bass.ts(m, 128)
                            for kp in range(KP):
                                nc.tensor.matmul(
                                    ps[:],
                                    wg_sb[:, 2 * kp : 2 * kp + 2, ms],
                                    g_t[:, 2 * kp : 2 * kp + 2, cs],
                                    start=(kp == 0),
                                    stop=False,
                                    perf_mode=mybir.MatmulPerfMode.DoubleRow,
                                )
                            for kc in range(KC):
                                nc.tensor.matmul(
                                    ps[:],
                                    wx_sb[:, kc, ms],
                                    x_t[:, kc, cs],
                                    start=False,
                                    stop=(kc == KC - 1),
                                )
                            rt = actp.tile([128, CHUNK], F16, tag=f"relu{m}")
                            nc.scalar.activation(
                                rt[:],
                                ps[:],
                                mybir.ActivationFunctionType.Relu,
                                bias=bgx_sb[:, m : m + 1],
                            )
                            relu_t.append(rt)
                        flush_pending()
                        if j % 2 == 0 and j > 0:
                            close_psi(x_t, o_t)
                            flush_epilog()
                        pending.append((relu_t, j))

                    flush_pending()
                    close_psi(x_t, o_t)
                    flush_epilog()
                    nc.scalar.dma_start(out_v[b, :, :, vs], o_t[:])

    nc.compile()
    return nc


def _prep_weights(Wg, bg, Wx, bx, Wpsi, bpsi_val):
    # wg[p, kc, m] = Wg[m, kc*128 + p] (stationary lhsT chunks; DoubleRow
    # matmuls consume [:, 2k:2k+2, ms] pairs)
    wg = np.ascontiguousarray(
        Wg.T.reshape(KC, 128, F_INT).transpose(1, 0, 2)
    ).astype(NP_F8)
    wx = np.ascontiguousarray(
        Wx.T.reshape(KC, 128, F_INT).transpose(1, 0, 2)
    ).astype(np.float16)
    # wpsi[p, m_chunk, :] = Wpsi[0, m_chunk*128 + p], replicated across all
    # 128 stationary columns so psi lands broadcast across partitions.
    wp = Wpsi[0].reshape(PSI_KC, 128).T
    wpsi = np.ascontiguousarray(np.repeat(wp[:, :, None], 128, axis=2)).astype(
        np.float16
    )
    bgx = np.empty((128, MC + 1), dtype=np.float32)
    bgx[:, :MC] = (bg + bx).reshape(MC, 128).T
    bgx[:, MC] = bpsi_val
    return wg, wx, wpsi, bgx


def kernel(g, x, Wg, bg, Wx, bx, Wpsi, bpsi, _trace=False):
    if "nc" not in _cache:
        _cache["nc"] = _build()
    nc = _cache["nc"]

    g = np.asarray(g, dtype=np.float32)
    x = np.asarray(x, dtype=np.float32)
    bpsi_val = float(np.asarray(bpsi).reshape(-1)[0])
    wg, wx, wpsi, bgx = _prep_weights(
        np.asarray(Wg, np.float32),
        np.asarray(bg, np.float32),
        np.asarray(Wx, np.float32),
        np.asarray(bx, np.float32),
        np.asarray(Wpsi, np.float32),
        bpsi_val,
    )
    in_maps = []
    for k in range(N_CORES):
        sl = slice(k * D_PER_CORE, (k + 1) * D_PER_CORE)
        in_maps.append(
            {
                "g": np.ascontiguousarray(g[:, :, sl])
                .reshape(B, C, V)
                .astype(NP_F8),
                "x": np.ascontiguousarray(x[:, :, sl])
                .reshape(B, C, V)
                .astype(np.float16),
                "wg": wg,
                "wx": wx,
                "wpsi": wpsi,
                "bgx": bgx,
            }
        )
    try:
        res = run_bass_kernel_spmd(nc, in_maps, list(range(N_CORES)), trace=_trace)
    except Exception:
        # transient axon/PJRT hiccups have been observed; one retry
        res = run_bass_kernel_spmd(nc, in_maps, list(range(N_CORES)), trace=_trace)

    out = np.empty((B, C, D, H, W), dtype=np.float32)
    for k in range(N_CORES):
        sl = slice(k * D_PER_CORE, (k + 1) * D_PER_CORE)
        out[:, :, sl] = (
            res.results[k]["out"].astype(np.float32).reshape(B, C, D_PER_CORE, H, W)
        )
    if _trace:
        return out, res
    return out
